# revision 3
# baseline (speedup 1.0000x reference)
"""Trainium2 Bass kernel for MatrixGraphConvolution (sparse gather-free).

out = D^-1 A (x @ W.T) + x @ B.T,  A[dst,src]=1 (set semantics),
deg counts duplicate edges, N=16384, E=524288, F=128.

Strategy (8 NeuronCores, row-sharded by dst):
  * The dense A^T stream of the previous kernel (32MB/core) is replaced
    by an edge-expanded x stream: host dedupes edges, sorts by dst, and
    materializes g[chunk, e, :] = x[src(e), :] fp16 for 128-edge chunks
    (~17MB/core).  Each chunk's dsts fall inside one aligned 128-dst
    PSUM window, so the segmented sum A@x becomes one 128-col matmul
    per chunk: psum_Y[:, win] += g_c^T @ S_c, with S_c the one-hot
    dst-offset matrix of the chunk.
  * S is built on-device on the otherwise-idle Vector engine:
    S[e, j, c] = (dstrel[e, c] == j) via one broadcast is_equal per
    32-chunk block (layout [128, j, c] so the matmul rhs is a strided
    column slice) - only the 1B-per-edge dstrel stream comes from HBM.
  * All 8 cores share one SPMD program: per-window chunk counts are
    equalized across cores (max over cores, zero-padded lanes).
  * Residual x @ B.T accumulates in a second PSUM region with x
    pre-scaled by deg on the host; per-window eviction (cast Y to fp16,
    apply W, scale by invdeg, DMA out) is pipelined into the main loop.
"""

import sys

sys.path.insert(0, "/opt/trn_rl_repo")

import numpy as np

import concourse.bass as bass
import concourse.tile as tile
import concourse.mybir as mybir
from concourse import bacc
from concourse.bass import ts, ds
from concourse.bass_utils import run_bass_kernel_spmd

N, E, F = 16384, 524288, 128
NCORES = 8
SH = N // NCORES          # 2048 dst rows per core
SHB = 11                  # log2(SH)
NWIN = SH // 128          # 16 psum windows of 128 dsts
BLK = 32                  # chunks per block (DMA/DVE granularity)

FP16 = mybir.dt.float16
FP32 = mybir.dt.float32

_NC = {}


def _build(cw: tuple):
    """cw[w] = chunks for window w (same for all cores)."""
    if cw in _NC:
        return _NC[cw]
    ncht = sum(cw)
    nblk = ncht // BLK
    assert ncht % BLK == 0
    base = np.concatenate([[0], np.cumsum(cw)])

    win = np.empty(ncht, np.int32)
    first = np.zeros(ncht, bool)
    last = np.zeros(ncht, bool)
    for w in range(NWIN):
        win[base[w] : base[w + 1]] = w
        first[base[w]] = True
        last[base[w + 1] - 1] = True

    nc = bacc.Bacc(None, target_bir_lowering=False)
    g = nc.dram_tensor("g", [nblk, 128, BLK * F], FP16, kind="ExternalInput")
    drel = nc.dram_tensor("drel", [128, ncht], FP16, kind="ExternalInput")
    iot = nc.dram_tensor("iot", [128, 128 * BLK], FP16, kind="ExternalInput")
    xtc = nc.dram_tensor("xtc", [F, SH], FP16, kind="ExternalInput")
    wt = nc.dram_tensor("wt", [F, F], FP16, kind="ExternalInput")
    bt = nc.dram_tensor("bt", [F, F], FP16, kind="ExternalInput")
    idr = nc.dram_tensor("idr", [1, SH], FP32, kind="ExternalInput")
    outT = nc.dram_tensor("outT", [F, SH], FP32, kind="ExternalOutput")

    with tile.TileContext(nc) as tc:
        with (
            tc.tile_pool(name="const", bufs=1) as constp,
            tc.tile_pool(name="gpool", bufs=3) as gpool,
            tc.tile_pool(name="spool", bufs=2) as spool,
            tc.tile_pool(name="psA", bufs=1, space=bass.MemorySpace.PSUM) as psA,
        ):
            drel_sb = constp.tile([128, ncht], FP16, tag="drel")
            nc.scalar.dma_start(drel_sb[:], drel[:])
            iot_sb = constp.tile([128, 128, BLK], FP16, tag="iot")
            nc.scalar.dma_start(iot_sb[:], iot[:])
            bt_sb = constp.tile([F, F], FP16, tag="bt")
            nc.scalar.dma_start(bt_sb[:], bt[:])
            wt_sb = constp.tile([F, F], FP16, tag="wt")
            nc.scalar.dma_start(wt_sb[:], wt[:])
            xtc_sb = constp.tile([F, SH], FP16, tag="xtc")
            nc.scalar.dma_start(xtc_sb[:], xtc[:])
            idr_sb = constp.tile([1, SH], FP32, tag="idr")
            nc.scalar.dma_start(idr_sb[:], idr[:])

            psy = psA.tile([128, SH], FP32, tag="y")    # Y = (A@x)^T
            ps = psA.tile([128, SH], FP32, tag="agg")   # res + W @ Y

            # invdeg row partition-broadcast on the idle GpSimd engine
            idb_sb = constp.tile([128, SH], FP32, tag="idb")
            nc.gpsimd.partition_broadcast(idb_sb[:], idr_sb[:])

            # residual: ps[f, n] = sum_c B^T[c, f] * (deg*x)^T[c, n]
            for b in range(SH // 512):
                nc.tensor.matmul(
                    ps[:, ts(b, 512)],
                    bt_sb[:],
                    xtc_sb[:, ts(b, 512)],
                    start=True,
                    stop=False,
                )

            y_sb = constp.tile([128, SH], FP16, tag="ysb")
            out_sb = constp.tile([128, SH], FP32, tag="osb")

            g_t = [None] * nblk
            s_t = [None] * nblk

            def load(blk):
                g_t[blk] = gpool.tile([128, BLK * F], FP16, tag="g", name=f"g{blk}")
                nc.sync.dma_start(g_t[blk][:], g[blk])

            def sbuild(blk):
                s_t[blk] = spool.tile(
                    [128, 128, BLK], FP16, tag="s", name=f"s{blk}"
                )
                d_b = (
                    drel_sb[:, ds(blk * BLK, BLK)]
                    .unsqueeze(1)
                    .broadcast_to([128, 128, BLK])
                )
                nc.vector.tensor_tensor(
                    s_t[blk][:], d_b, iot_sb[:], op=mybir.AluOpType.is_equal
                )

            load(0)
            sbuild(0)
            load(1)

            pending = []
            for blk in range(nblk):
                # evict windows finished in the previous block: cast first
                # (DVE) so the PE-side W-matmul's wait is short
                for w in pending:
                    nc.vector.tensor_copy(
                        y_sb[:, ds(w * 128, 128)], psy[:, ds(w * 128, 128)]
                    )
                if blk + 1 < nblk:
                    sbuild(blk + 1)
                if blk + 2 < nblk:
                    load(blk + 2)
                for w in pending:
                    nc.tensor.matmul(
                        ps[:, ds(w * 128, 128)],
                        wt_sb[:],
                        y_sb[:, ds(w * 128, 128)],
                        start=False,
                        stop=True,
                    )
                for w in pending:
                    nc.vector.tensor_mul(
                        out_sb[:, ds(w * 128, 128)],
                        ps[:, ds(w * 128, 128)],
                        idb_sb[:, ds(w * 128, 128)],
                    )
                    nc.scalar.dma_start(
                        outT[:, ds(w * 128, 128)], out_sb[:, ds(w * 128, 128)]
                    )
                pending = []
                for cl in range(BLK):
                    c = blk * BLK + cl
                    w = int(win[c])
                    nc.tensor.matmul(
                        psy[:, ds(w * 128, 128)],
                        g_t[blk][:, ts(cl, F)],
                        s_t[blk][:, :, cl],
                        start=bool(first[c]),
                        stop=bool(last[c]),
                    )
                    if last[c]:
                        pending.append(w)
                g_t[blk] = None
                s_t[blk] = None

            # tail: evict the remaining windows
            for w in pending:
                nc.vector.tensor_copy(
                    y_sb[:, ds(w * 128, 128)], psy[:, ds(w * 128, 128)]
                )
                nc.tensor.matmul(
                    ps[:, ds(w * 128, 128)],
                    wt_sb[:],
                    y_sb[:, ds(w * 128, 128)],
                    start=False,
                    stop=True,
                )
                nc.vector.tensor_mul(
                    out_sb[:, ds(w * 128, 128)],
                    ps[:, ds(w * 128, 128)],
                    idb_sb[:, ds(w * 128, 128)],
                )
                nc.scalar.dma_start(
                    outT[:, ds(w * 128, 128)], out_sb[:, ds(w * 128, 128)]
                )

    nc.compile()
    _NC[cw] = nc
    return nc


def _prep_inputs(x, edge_index, W, B):
    src = np.asarray(edge_index[0]).astype(np.int64)
    dst = np.asarray(edge_index[1]).astype(np.int64)
    x = np.asarray(x, dtype=np.float32)
    W = np.asarray(W, dtype=np.float32)
    B = np.asarray(B, dtype=np.float32)

    deg = np.bincount(dst, minlength=N).astype(np.float32)
    dtil = np.where(deg == 0, np.float32(1.0), deg)
    invdeg = (np.float32(1.0) / dtil).astype(np.float32)

    # set semantics: dedupe (dst, src) pairs; unique() also sorts by dst
    keys = np.unique(dst * N + src)
    udst = (keys // N).astype(np.int64)
    usrc = (keys % N).astype(np.int64)

    ucore = (udst >> SHB).astype(np.int64)
    uwin = ((udst & (SH - 1)) >> 7).astype(np.int64)
    udrel = (udst & 127).astype(np.int64)

    # per (core, window) edge counts -> shared schedule
    cnt = np.bincount(ucore * NWIN + uwin, minlength=NCORES * NWIN).reshape(
        NCORES, NWIN
    )
    cw = np.maximum((cnt.max(axis=0) + 127) // 128, 1)
    rem = (-cw.sum()) % BLK
    cw[NWIN - 1] += rem
    cw = tuple(int(v) for v in cw)
    ncht = sum(cw)
    nblk = ncht // BLK
    base = np.concatenate([[0], np.cumsum(cw)]).astype(np.int64)

    x16 = x.astype(np.float16)
    xtil = (dtil[:, None] * x).astype(np.float16)
    wt_np = np.ascontiguousarray(W.T).astype(np.float16)
    bt_np = np.ascontiguousarray(B.T).astype(np.float16)
    iot_np = np.ascontiguousarray(
        np.broadcast_to(
            np.arange(128, dtype=np.float16)[None, :, None], (128, 128, BLK)
        ).reshape(128, 128 * BLK)
    )

    # edge -> (chunk, lane): edges are sorted by dst, so within each
    # (core, window) group they are consecutive; local index = position
    # minus group start
    grp = ucore * NWIN + uwin
    grp_start_pos = np.concatenate([[0], np.cumsum(np.bincount(grp, minlength=NCORES * NWIN))])
    pos = np.arange(len(udst), dtype=np.int64)
    loc = pos - grp_start_pos[grp]
    chunk = base[uwin] + (loc >> 7)          # chunk id within the core's stream
    lane = loc & 127

    in_maps = []
    for k in range(NCORES):
        m = ucore == k
        g_flat = np.zeros((ncht, 128, F), dtype=np.float16)
        g_flat[chunk[m], lane[m], :] = x16[usrc[m]]
        g_np = np.ascontiguousarray(
            g_flat.reshape(nblk, BLK, 128, F).transpose(0, 2, 1, 3)
        ).reshape(nblk, 128, BLK * F)
        drel_np = np.zeros((128, ncht), dtype=np.float16)
        drel_np[lane[m], chunk[m]] = udrel[m].astype(np.float16)
        sl = slice(k * SH, (k + 1) * SH)
        in_maps.append(
            {
                "g": g_np,
                "drel": drel_np,
                "iot": iot_np,
                "xtc": np.ascontiguousarray(xtil[sl].T),
                "wt": wt_np,
                "bt": bt_np,
                "idr": np.ascontiguousarray(invdeg[None, sl]),
            }
        )
    return cw, in_maps


def kernel(x, edge_index, W, B):
    cw, in_maps = _prep_inputs(x, edge_index, W, B)
    nc = _build(cw)
    res = run_bass_kernel_spmd(nc, in_maps, core_ids=list(range(NCORES)))
    out = np.empty((N, F), dtype=np.float32)
    for k in range(NCORES):
        out[k * SH : (k + 1) * SH, :] = res.results[k]["outT"].T
    return out


# revision 5
# speedup vs baseline: 1.7892x; 1.7892x over previous
"""Trainium2 Bass kernel for MatrixGraphConvolution (sparse gather-free).

out = D^-1 A (x @ W.T) + x @ B.T,  A[dst,src]=1 (set semantics),
deg counts duplicate edges, N=16384, E=524288, F=128.

Strategy (8 NeuronCores, row-sharded by dst):
  * The dense A^T stream (32MB/core) is replaced by an edge-expanded x
    stream: host dedupes edges, sorts by dst, and materializes
    g[chunk, e, :] = x[src(e), :] fp16 for 128-edge chunks (~18MB/core).
    Each chunk's dsts fall inside one aligned 64-dst PSUM window, so the
    segmented sum A@x becomes one 64-col matmul per chunk:
    psum_Y[:, win] += g_c^T @ S_c, with S_c the one-hot dst-offset
    matrix of the chunk.  Per-chunk cost is LDWEIGHTS-dominated
    (~100ns), so the moving side is kept narrow (W=64) and S is laid
    out c-major so the matmul rhs is contiguous (strided rhs measured
    2x slower).
  * Chunks are scheduled in window quads that rotate across the 4 PSUM
    banks (consecutive matmuls never hit the same bank).
  * S is built on-device on the otherwise-idle Vector engine:
    S[e, c, j] = (dstrel[e, c] == iota[j]) via broadcast is_equal per
    16-chunk block - only the 1B-per-edge dstrel stream comes from HBM.
  * All 8 cores share one SPMD program: per-window chunk counts are
    equalized across cores (max over cores, zero-padded lanes).
  * Residual x @ B.T accumulates in a second PSUM region with x
    pre-scaled by deg on the host; per-window eviction (cast Y to fp16,
    apply W, scale by invdeg, DMA out) is pipelined into the main loop.
"""

import sys

sys.path.insert(0, "/opt/trn_rl_repo")

import numpy as np

import concourse.bass as bass
import concourse.tile as tile
import concourse.mybir as mybir
from concourse import bacc
from concourse.bass import ts, ds
from concourse.bass_utils import run_bass_kernel_spmd

N, E, F = 16384, 524288, 128
NCORES = 8
SH = N // NCORES          # 2048 dst rows per core
SHB = 11                  # log2(SH)
WW = 64                   # psum window width (dsts per window)
NWIN = SH // WW           # 32 windows per core
NQ = NWIN // 4            # window quads (one window per psum bank)
BLK = 16                  # chunks per block (DMA/DVE granularity)

FP16 = mybir.dt.float16
FP32 = mybir.dt.float32

_NC = {}


def _schedule(cw):
    """Quad round-robin chunk order. cw[w] = chunks for window w.
    Returns (win, first, last) per chunk position."""
    order = []
    for q in range(NQ):
        quad = [q, NQ + q, 2 * NQ + q, 3 * NQ + q]
        rem = {w: cw[w] for w in quad}
        li = {w: 0 for w in quad}
        while any(rem.values()):
            for w in quad:
                if rem[w]:
                    order.append((w, li[w]))
                    li[w] += 1
                    rem[w] -= 1
    win = np.array([w for w, _ in order], np.int32)
    first = np.array([l == 0 for w, l in order], bool)
    last = np.array([l == cw[w] - 1 for w, l in order], bool)
    return order, win, first, last


def _build(cw: tuple):
    if cw in _NC:
        return _NC[cw]
    ncht = sum(cw)
    assert ncht % BLK == 0
    nblk = ncht // BLK
    _, win, first, last = _schedule(cw)

    nc = bacc.Bacc(None, target_bir_lowering=False)
    g = nc.dram_tensor("g", [nblk, 128, BLK * F], FP16, kind="ExternalInput")
    drel = nc.dram_tensor("drel", [128, ncht], FP16, kind="ExternalInput")
    iot = nc.dram_tensor("iot", [128, WW], FP16, kind="ExternalInput")
    xtc = nc.dram_tensor("xtc", [F, SH], FP16, kind="ExternalInput")
    wt = nc.dram_tensor("wt", [F, F], FP16, kind="ExternalInput")
    bt = nc.dram_tensor("bt", [F, F], FP16, kind="ExternalInput")
    idr = nc.dram_tensor("idr", [1, SH], FP32, kind="ExternalInput")
    outT = nc.dram_tensor("outT", [F, SH], FP32, kind="ExternalOutput")

    with tile.TileContext(nc) as tc:
        with (
            tc.tile_pool(name="const", bufs=1) as constp,
            tc.tile_pool(name="gpool", bufs=4) as gpool,
            tc.tile_pool(name="spool", bufs=3) as spool,
            tc.tile_pool(name="psA", bufs=1, space=bass.MemorySpace.PSUM) as psA,
        ):
            drel_sb = constp.tile([128, ncht], FP16, tag="drel")
            nc.scalar.dma_start(drel_sb[:], drel[:])
            iot_sb = constp.tile([128, WW], FP16, tag="iot")
            nc.scalar.dma_start(iot_sb[:], iot[:])
            bt_sb = constp.tile([F, F], FP16, tag="bt")
            nc.scalar.dma_start(bt_sb[:], bt[:])
            wt_sb = constp.tile([F, F], FP16, tag="wt")
            nc.scalar.dma_start(wt_sb[:], wt[:])
            xtc_sb = constp.tile([F, SH], FP16, tag="xtc")
            nc.scalar.dma_start(xtc_sb[:], xtc[:])
            idr_sb = constp.tile([1, SH], FP32, tag="idr")
            nc.scalar.dma_start(idr_sb[:], idr[:])

            psy = psA.tile([128, SH], FP32, tag="y")    # Y = (A@x)^T
            ps = psA.tile([128, SH], FP32, tag="agg")   # res + W @ Y

            # invdeg row partition-broadcast on the idle GpSimd engine
            idb_sb = constp.tile([128, SH], FP32, tag="idb")
            nc.gpsimd.partition_broadcast(idb_sb[:], idr_sb[:])

            y_sb = constp.tile([128, SH], FP16, tag="ysb")
            out_sb = constp.tile([128, SH], FP32, tag="osb")

            g_t = [None] * nblk
            s_t = [None] * nblk

            def load(blk):
                g_t[blk] = gpool.tile([128, BLK * F], FP16, tag="g", name=f"g{blk}")
                nc.sync.dma_start(g_t[blk][:], g[blk])

            def sbuild(blk):
                s_t[blk] = spool.tile(
                    [128, BLK, WW], FP16, tag="s", name=f"s{blk}"
                )
                d_b = (
                    drel_sb[:, ds(blk * BLK, BLK)]
                    .unsqueeze(2)
                    .broadcast_to([128, BLK, WW])
                )
                i_b = iot_sb[:].unsqueeze(1).broadcast_to([128, BLK, WW])
                nc.vector.tensor_tensor(
                    s_t[blk][:], d_b, i_b, op=mybir.AluOpType.is_equal
                )

            load(0)
            sbuild(0)
            load(1)
            sbuild(1)
            load(2)

            pending = []
            for blk in range(nblk):
                # evict windows finished in the previous block: cast first
                # (DVE) so the PE-side W-matmul's wait is short
                for w in pending:
                    nc.vector.tensor_copy(
                        y_sb[:, ds(w * WW, WW)], psy[:, ds(w * WW, WW)]
                    )
                if blk + 2 < nblk:
                    sbuild(blk + 2)
                if blk + 3 < nblk:
                    load(blk + 3)
                for w in pending:
                    nc.tensor.matmul(
                        ps[:, ds(w * WW, WW)],
                        wt_sb[:],
                        y_sb[:, ds(w * WW, WW)],
                        start=False,
                        stop=True,
                    )
                for w in pending:
                    nc.vector.tensor_mul(
                        out_sb[:, ds(w * WW, WW)],
                        ps[:, ds(w * WW, WW)],
                        idb_sb[:, ds(w * WW, WW)],
                    )
                    nc.scalar.dma_start(
                        outT[:, ds(w * WW, WW)], out_sb[:, ds(w * WW, WW)]
                    )
                pending = []
                for cl in range(BLK):
                    c = blk * BLK + cl
                    w = int(win[c])
                    nc.tensor.matmul(
                        psy[:, ds(w * WW, WW)],
                        g_t[blk][:, ts(cl, F)],
                        s_t[blk][:, cl, :],
                        start=bool(first[c]),
                        stop=bool(last[c]),
                    )
                    if last[c]:
                        pending.append(w)
                if blk == 0:
                    # residual ps[f, n] = sum_c B^T[c, f] * (deg*x)^T[c, n];
                    # emitted after the first block so it does not gate the
                    # pipelined start
                    for b in range(SH // 512):
                        nc.tensor.matmul(
                            ps[:, ts(b, 512)],
                            bt_sb[:],
                            xtc_sb[:, ts(b, 512)],
                            start=True,
                            stop=False,
                        )
                g_t[blk] = None
                s_t[blk] = None

            # tail: evict the remaining windows
            for w in pending:
                nc.vector.tensor_copy(
                    y_sb[:, ds(w * WW, WW)], psy[:, ds(w * WW, WW)]
                )
                nc.tensor.matmul(
                    ps[:, ds(w * WW, WW)],
                    wt_sb[:],
                    y_sb[:, ds(w * WW, WW)],
                    start=False,
                    stop=True,
                )
                nc.vector.tensor_mul(
                    out_sb[:, ds(w * WW, WW)],
                    ps[:, ds(w * WW, WW)],
                    idb_sb[:, ds(w * WW, WW)],
                )
                nc.scalar.dma_start(
                    outT[:, ds(w * WW, WW)], out_sb[:, ds(w * WW, WW)]
                )

    nc.compile()
    _NC[cw] = nc
    return nc


def _prep_inputs(x, edge_index, W, B):
    src = np.asarray(edge_index[0]).astype(np.int64)
    dst = np.asarray(edge_index[1]).astype(np.int64)
    x = np.asarray(x, dtype=np.float32)
    Wm = np.asarray(W, dtype=np.float32)
    B = np.asarray(B, dtype=np.float32)

    deg = np.bincount(dst, minlength=N).astype(np.float32)
    dtil = np.where(deg == 0, np.float32(1.0), deg)
    invdeg = (np.float32(1.0) / dtil).astype(np.float32)

    # set semantics: dedupe (dst, src) pairs; unique() also sorts by dst
    keys = np.unique(dst * N + src)
    udst = (keys // N).astype(np.int64)
    usrc = (keys % N).astype(np.int64)

    ucore = (udst >> SHB).astype(np.int64)
    uwin = ((udst & (SH - 1)) // WW).astype(np.int64)
    udrel = (udst % WW).astype(np.int64)

    # per (core, window) edge counts -> shared schedule
    cnt = np.bincount(ucore * NWIN + uwin, minlength=NCORES * NWIN).reshape(
        NCORES, NWIN
    )
    cw = np.maximum((cnt.max(axis=0) + 127) // 128, 1)
    cw[NWIN - 1] += (-cw.sum()) % BLK
    cw = tuple(int(v) for v in cw)
    ncht = sum(cw)
    nblk = ncht // BLK

    order, _, _, _ = _schedule(cw)
    cwmax = max(cw)
    chunkpos = np.full((NWIN, cwmax), -1, np.int64)
    for pos, (w, l) in enumerate(order):
        chunkpos[w, l] = pos

    x16 = x.astype(np.float16)
    xtil = (dtil[:, None] * x).astype(np.float16)
    wt_np = np.ascontiguousarray(Wm.T).astype(np.float16)
    bt_np = np.ascontiguousarray(B.T).astype(np.float16)
    iot_np = np.ascontiguousarray(
        np.broadcast_to(np.arange(WW, dtype=np.float16)[None, :], (128, WW))
    )

    # edge -> (chunk, lane): edges are sorted by dst, so within each
    # (core, window) group they are consecutive
    grp = ucore * NWIN + uwin
    grp_start = np.concatenate(
        [[0], np.cumsum(np.bincount(grp, minlength=NCORES * NWIN))]
    )
    loc = np.arange(len(udst), dtype=np.int64) - grp_start[grp]
    chunk = chunkpos[uwin, loc >> 7]
    lane = loc & 127

    in_maps = []
    for k in range(NCORES):
        m = ucore == k
        g_flat = np.zeros((ncht, 128, F), dtype=np.float16)
        g_flat[chunk[m], lane[m], :] = x16[usrc[m]]
        g_np = np.ascontiguousarray(
            g_flat.reshape(nblk, BLK, 128, F).transpose(0, 2, 1, 3)
        ).reshape(nblk, 128, BLK * F)
        drel_np = np.zeros((128, ncht), dtype=np.float16)
        drel_np[lane[m], chunk[m]] = udrel[m].astype(np.float16)
        sl = slice(k * SH, (k + 1) * SH)
        in_maps.append(
            {
                "g": g_np,
                "drel": drel_np,
                "iot": iot_np,
                "xtc": np.ascontiguousarray(xtil[sl].T),
                "wt": wt_np,
                "bt": bt_np,
                "idr": np.ascontiguousarray(invdeg[None, sl]),
            }
        )
    return cw, in_maps


def kernel(x, edge_index, W, B):
    cw, in_maps = _prep_inputs(x, edge_index, W, B)
    nc = _build(cw)
    res = run_bass_kernel_spmd(nc, in_maps, core_ids=list(range(NCORES)))
    out = np.empty((N, F), dtype=np.float32)
    for k in range(NCORES):
        out[k * SH : (k + 1) * SH, :] = res.results[k]["outT"].T
    return out


# revision 6
# speedup vs baseline: 2.0328x; 1.1362x over previous
"""Trainium2 Bass kernel for MatrixGraphConvolution (sparse gather-free).

out = D^-1 A (x @ W.T) + x @ B.T,  A[dst,src]=1 (set semantics),
deg counts duplicate edges, N=16384, E=524288, F=128.

Strategy (8 NeuronCores, row-sharded by dst):
  * The dense A^T stream (32MB/core) is replaced by an edge-expanded x
    stream: host dedupes edges, sorts by dst, and materializes
    g[chunk, e, :] = x[src(e), :] fp16 for 128-edge chunks (~18MB/core).
    Each chunk's dsts fall inside one aligned 64-dst PSUM window, so the
    segmented sum A@x becomes one 64-col matmul per chunk:
    psum_Y[:, win] += g_c^T @ S_c, with S_c the one-hot dst-offset
    matrix of the chunk.  Per-chunk cost is LDWEIGHTS-dominated, so the
    moving side is kept narrow (W=64) and S is laid out c-major so the
    matmul rhs is contiguous (strided rhs measured 2x slower).
  * Chunks are scheduled in window quads that rotate across the 4 PSUM
    banks (consecutive matmuls never hit the same bank).
  * S is built on-device on the otherwise-idle Vector engine:
    S[e, c, j] = (dstrel[e, c] == iota[j]) via broadcast is_equal per
    32-chunk block - only the 1B-per-edge dstrel stream comes from HBM.
  * All 8 cores share one SPMD program: per-window chunk counts are
    equalized across cores (max over cores, zero-padded lanes).
  * Residual x @ B.T accumulates in a second PSUM region with x
    pre-scaled by deg on the host; per-window eviction (Act-engine cast
    of Y to fp16, apply W, scale by invdeg, DMA out) is pipelined into
    the main loop.  Evicted windows are written to outT in eviction
    order (contiguous slots -> batched 1KB-desc DMAs); the host
    un-permutes the columns.
"""

import sys

sys.path.insert(0, "/opt/trn_rl_repo")

import numpy as np

import concourse.bass as bass
import concourse.tile as tile
import concourse.mybir as mybir
from concourse import bacc
from concourse.bass import ts, ds
from concourse.bass_utils import run_bass_kernel_spmd

N, E, F = 16384, 524288, 128
NCORES = 8
SH = N // NCORES          # 2048 dst rows per core
SHB = 11                  # log2(SH)
WW = 64                   # psum window width (dsts per window)
NWIN = SH // WW           # 32 windows per core
NQ = NWIN // 4            # window quads (one window per psum bank)
BLK = 32                  # chunks per block (DMA/DVE granularity)

FP16 = mybir.dt.float16
FP32 = mybir.dt.float32

_NC = {}


def _schedule(cw):
    """Quad round-robin chunk order. cw[w] = chunks for window w."""
    order = []
    for q in range(NQ):
        quad = [q, NQ + q, 2 * NQ + q, 3 * NQ + q]
        rem = {w: cw[w] for w in quad}
        li = {w: 0 for w in quad}
        while any(rem.values()):
            for w in quad:
                if rem[w]:
                    order.append((w, li[w]))
                    li[w] += 1
                    rem[w] -= 1
    win = np.array([w for w, _ in order], np.int32)
    first = np.array([l == 0 for w, l in order], bool)
    last = np.array([l == cw[w] - 1 for w, l in order], bool)
    # eviction order: windows sorted by the position of their last chunk
    lastpos = {}
    for pos, (w, l) in enumerate(order):
        if l == cw[w] - 1:
            lastpos[w] = pos
    evict_order = sorted(range(NWIN), key=lambda w: lastpos[w])
    return order, win, first, last, evict_order


def _build(cw: tuple):
    if cw in _NC:
        return _NC[cw]
    ncht = sum(cw)
    assert ncht % BLK == 0
    nblk = ncht // BLK
    _, win, first, last, evict_order = _schedule(cw)
    qpos = {w: i for i, w in enumerate(evict_order)}
    first_last = int(np.nonzero(last)[0][0])
    resid_blk = 0 if first_last < BLK else 1

    nc = bacc.Bacc(None, target_bir_lowering=False)
    g = nc.dram_tensor("g", [nblk, 128, BLK * F], FP16, kind="ExternalInput")
    drel = nc.dram_tensor("drel", [128, ncht], FP16, kind="ExternalInput")
    iot = nc.dram_tensor("iot", [128, WW], FP16, kind="ExternalInput")
    xtc = nc.dram_tensor("xtc", [F, SH], FP16, kind="ExternalInput")
    wt = nc.dram_tensor("wt", [F, F], FP16, kind="ExternalInput")
    bt = nc.dram_tensor("bt", [F, F], FP16, kind="ExternalInput")
    idr = nc.dram_tensor("idr", [1, SH], FP32, kind="ExternalInput")
    outT = nc.dram_tensor("outT", [F, SH], FP32, kind="ExternalOutput")

    with tile.TileContext(nc) as tc:
        with (
            tc.tile_pool(name="const", bufs=1) as constp,
            tc.tile_pool(name="gpool", bufs=4) as gpool,
            tc.tile_pool(name="spool", bufs=3) as spool,
            tc.tile_pool(name="psA", bufs=1, space=bass.MemorySpace.PSUM) as psA,
        ):
            # small consts on the scalar ring
            iot_sb = constp.tile([128, WW], FP16, tag="iot")
            nc.scalar.dma_start(iot_sb[:], iot[:])
            bt_sb = constp.tile([F, F], FP16, tag="bt")
            nc.scalar.dma_start(bt_sb[:], bt[:])
            wt_sb = constp.tile([F, F], FP16, tag="wt")
            nc.scalar.dma_start(wt_sb[:], wt[:])
            idr_sb = constp.tile([1, SH], FP32, tag="idr")
            nc.scalar.dma_start(idr_sb[:], idr[:])

            psy = psA.tile([128, SH], FP32, tag="y")    # Y = (A@x)^T
            ps = psA.tile([128, SH], FP32, tag="agg")   # res + W @ Y

            # invdeg row partition-broadcast on the idle GpSimd engine
            idb_sb = constp.tile([128, SH], FP32, tag="idb")
            nc.gpsimd.partition_broadcast(idb_sb[:], idr_sb[:])

            y_sb = constp.tile([128, SH], FP16, tag="ysb")
            out_sb = constp.tile([128, SH], FP32, tag="osb")

            g_t = [None] * nblk
            s_t = [None] * nblk

            def load(blk):
                g_t[blk] = gpool.tile([128, BLK * F], FP16, tag="g", name=f"g{blk}")
                nc.sync.dma_start(g_t[blk][:], g[blk])

            def sbuild(blk):
                s_t[blk] = spool.tile(
                    [128, BLK, WW], FP16, tag="s", name=f"s{blk}"
                )
                d_b = (
                    drel_sb[:, ds(blk * BLK, BLK)]
                    .unsqueeze(2)
                    .broadcast_to([128, BLK, WW])
                )
                i_b = iot_sb[:].unsqueeze(1).broadcast_to([128, BLK, WW])
                nc.vector.tensor_tensor(
                    s_t[blk][:], d_b, i_b, op=mybir.AluOpType.is_equal
                )

            # big streams on the sync HWDGE ring: drel first (gates S),
            # then the first g blocks, then xtc (only gates the residual)
            drel_sb = constp.tile([128, ncht], FP16, tag="drel")
            nc.sync.dma_start(drel_sb[:], drel[:])
            load(0)
            sbuild(0)
            load(1)
            sbuild(1)
            load(2)
            xtc_sb = constp.tile([F, SH], FP16, tag="xtc")
            nc.sync.dma_start(xtc_sb[:], xtc[:])

            def evict(pending):
                if not pending:
                    return
                q0 = qpos[pending[0]]
                for w in pending:
                    nc.scalar.copy(
                        y_sb[:, ds(w * WW, WW)], psy[:, ds(w * WW, WW)]
                    )
                for w in pending:
                    nc.tensor.matmul(
                        ps[:, ds(w * WW, WW)],
                        wt_sb[:],
                        y_sb[:, ds(w * WW, WW)],
                        start=False,
                        stop=True,
                    )
                for w in pending:
                    nc.vector.tensor_mul(
                        out_sb[:, ds(qpos[w] * WW, WW)],
                        ps[:, ds(w * WW, WW)],
                        idb_sb[:, ds(w * WW, WW)],
                    )
                nc.scalar.dma_start(
                    outT[:, ds(q0 * WW, len(pending) * WW)],
                    out_sb[:, ds(q0 * WW, len(pending) * WW)],
                )

            pending = []
            for blk in range(nblk):
                ev, pending = pending, []
                for w in ev:
                    nc.scalar.copy(
                        y_sb[:, ds(w * WW, WW)], psy[:, ds(w * WW, WW)]
                    )
                if blk + 2 < nblk:
                    sbuild(blk + 2)
                if blk + 3 < nblk:
                    load(blk + 3)
                for w in ev:
                    nc.tensor.matmul(
                        ps[:, ds(w * WW, WW)],
                        wt_sb[:],
                        y_sb[:, ds(w * WW, WW)],
                        start=False,
                        stop=True,
                    )
                for w in ev:
                    nc.vector.tensor_mul(
                        out_sb[:, ds(qpos[w] * WW, WW)],
                        ps[:, ds(w * WW, WW)],
                        idb_sb[:, ds(w * WW, WW)],
                    )
                if ev:
                    q0 = qpos[ev[0]]
                    nc.scalar.dma_start(
                        outT[:, ds(q0 * WW, len(ev) * WW)],
                        out_sb[:, ds(q0 * WW, len(ev) * WW)],
                    )
                for cl in range(BLK):
                    c = blk * BLK + cl
                    w = int(win[c])
                    nc.tensor.matmul(
                        psy[:, ds(w * WW, WW)],
                        g_t[blk][:, ts(cl, F)],
                        s_t[blk][:, cl, :],
                        start=bool(first[c]),
                        stop=bool(last[c]),
                    )
                    if last[c]:
                        pending.append(w)
                if blk == resid_blk:
                    # residual ps[f, n] = sum_c B^T[c, f] * (deg*x)^T[c, n];
                    # emitted after the first block(s) so it does not gate
                    # the pipelined start
                    for b in range(SH // 512):
                        nc.tensor.matmul(
                            ps[:, ts(b, 512)],
                            bt_sb[:],
                            xtc_sb[:, ts(b, 512)],
                            start=True,
                            stop=False,
                        )
                g_t[blk] = None
                s_t[blk] = None

            evict(pending)

    nc.compile()
    _NC[cw] = nc
    return nc


def _prep_inputs(x, edge_index, W, B):
    src = np.asarray(edge_index[0]).astype(np.int64)
    dst = np.asarray(edge_index[1]).astype(np.int64)
    x = np.asarray(x, dtype=np.float32)
    Wm = np.asarray(W, dtype=np.float32)
    B = np.asarray(B, dtype=np.float32)

    deg = np.bincount(dst, minlength=N).astype(np.float32)
    dtil = np.where(deg == 0, np.float32(1.0), deg)
    invdeg = (np.float32(1.0) / dtil).astype(np.float32)

    # set semantics: dedupe (dst, src) pairs; unique() also sorts by dst
    keys = np.unique(dst * N + src)
    udst = (keys // N).astype(np.int64)
    usrc = (keys % N).astype(np.int64)

    ucore = (udst >> SHB).astype(np.int64)
    uwin = ((udst & (SH - 1)) // WW).astype(np.int64)
    udrel = (udst % WW).astype(np.int64)

    # per (core, window) edge counts -> shared schedule
    cnt = np.bincount(ucore * NWIN + uwin, minlength=NCORES * NWIN).reshape(
        NCORES, NWIN
    )
    cw = np.maximum((cnt.max(axis=0) + 127) // 128, 1)
    cw[NWIN - 1] += (-cw.sum()) % BLK
    cw = tuple(int(v) for v in cw)
    ncht = sum(cw)
    nblk = ncht // BLK

    order, _, _, _, evict_order = _schedule(cw)
    cwmax = max(cw)
    chunkpos = np.full((NWIN, cwmax), -1, np.int64)
    for pos, (w, l) in enumerate(order):
        chunkpos[w, l] = pos

    x16 = x.astype(np.float16)
    xtil = (dtil[:, None] * x).astype(np.float16)
    wt_np = np.ascontiguousarray(Wm.T).astype(np.float16)
    bt_np = np.ascontiguousarray(B.T).astype(np.float16)
    iot_np = np.ascontiguousarray(
        np.broadcast_to(np.arange(WW, dtype=np.float16)[None, :], (128, WW))
    )

    # edge -> (chunk, lane): edges are sorted by dst, so within each
    # (core, window) group they are consecutive
    grp = ucore * NWIN + uwin
    grp_start = np.concatenate(
        [[0], np.cumsum(np.bincount(grp, minlength=NCORES * NWIN))]
    )
    loc = np.arange(len(udst), dtype=np.int64) - grp_start[grp]
    chunk = chunkpos[uwin, loc >> 7]
    lane = loc & 127

    in_maps = []
    for k in range(NCORES):
        m = ucore == k
        g_flat = np.zeros((ncht, 128, F), dtype=np.float16)
        g_flat[chunk[m], lane[m], :] = x16[usrc[m]]
        g_np = np.ascontiguousarray(
            g_flat.reshape(nblk, BLK, 128, F).transpose(0, 2, 1, 3)
        ).reshape(nblk, 128, BLK * F)
        drel_np = np.zeros((128, ncht), dtype=np.float16)
        drel_np[lane[m], chunk[m]] = udrel[m].astype(np.float16)
        sl = slice(k * SH, (k + 1) * SH)
        in_maps.append(
            {
                "g": g_np,
                "drel": drel_np,
                "iot": iot_np,
                "xtc": np.ascontiguousarray(xtil[sl].T),
                "wt": wt_np,
                "bt": bt_np,
                "idr": np.ascontiguousarray(invdeg[None, sl]),
            }
        )
    # column un-permutation: outT slot qpos*WW+j holds dst evict_order[qpos]*WW+j
    colperm = np.concatenate(
        [np.arange(w * WW, (w + 1) * WW) for w in evict_order]
    )
    return cw, in_maps, colperm


def kernel(x, edge_index, W, B):
    cw, in_maps, colperm = _prep_inputs(x, edge_index, W, B)
    nc = _build(cw)
    res = run_bass_kernel_spmd(nc, in_maps, core_ids=list(range(NCORES)))
    out = np.empty((N, F), dtype=np.float32)
    for k in range(NCORES):
        out[k * SH + colperm, :] = res.results[k]["outT"].T
    return out
